# revision 16
# baseline (speedup 1.0000x reference)
"""LSTM decoder (nn_Decoder) on 8 Trainium2 NeuronCores.

Sharding:
  - LSTM gates [B,4H] split across cores: core k computes a 512-col slice
    (128 cols of each gate, order [g|i|f|o]) and its 128-col h-chunk.
  - Per-step 8-core AllGather of transposed h chunks (bf16) rebuilds full
    h.T (the lhsT for the next step's recurrent matmul and for the fc).
  - fc [H,V] sharded over V: core k computes 1280 padded vocab cols.
    fc work is split into 512-col PSUM groups and scheduled one or two
    per AllGather window so the PE stays busy (and HAM-warm) while the
    collective is in flight.
  - Each step's x-part gate contributions (embedding @ W_ih + bias) are
    matmul-accumulated into PSUM two steps ahead, also inside windows.
  - Embedding rows are gathered on device (indirect DMA), PE-transposed
    into x.T layout, pipelined a few steps ahead of use.
Weights/lhsT/exchange in bf16 (validated: adds <1e-3 abs error), PSUM
accumulation fp32.
"""
import sys
import numpy as np

if '/opt/trn_rl_repo' not in sys.path:
    sys.path.insert(0, '/opt/trn_rl_repo')

N_CORES = 8
B, T, E, H, V, FEAT = 64, 32, 512, 1024, 10000, 2048
G = 512            # per-core gate-slice width (4 gates x 128)
HC = 128           # per-core h chunk width
VL = 1280          # per-core padded vocab slice (10240/8)
KE, KH, KF = E // 128, H // 128, FEAT // 128
NG = (T * B) // 128  # embedding gather groups of 128 rows

_built = None
last_results = None


def _build():
    import concourse.bacc as bacc
    import concourse.bass as bass
    import concourse.mybir as mybir
    import concourse.tile as tile

    dt = mybir.dt
    f32, f32r, bf16, i32 = dt.float32, dt.float32r, dt.bfloat16, dt.int32
    SIG = mybir.ActivationFunctionType.Sigmoid
    TANH = mybir.ActivationFunctionType.Tanh
    MUL = mybir.AluOpType.mult
    RG = [list(range(N_CORES))]

    nc = bacc.Bacc("TRN2", target_bir_lowering=False, debug=False,
                   num_devices=N_CORES)

    emb_tab = nc.dram_tensor("emb_tab", [V, E], f32r, kind="ExternalInput")
    idx_g = nc.dram_tensor("idx_g", [128, NG], i32, kind="ExternalInput")
    featT = nc.dram_tensor("featT", [128, KF * B], bf16, kind="ExternalInput")
    w_init_h = nc.dram_tensor("w_init_h", [128, KF * HC], bf16, kind="ExternalInput")
    w_init_c = nc.dram_tensor("w_init_c", [128, KF * HC], bf16, kind="ExternalInput")
    b_init_h = nc.dram_tensor("b_init_h", [1, HC], bf16, kind="ExternalInput")
    b_init_c = nc.dram_tensor("b_init_c", [1, HC], bf16, kind="ExternalInput")
    w_ih = nc.dram_tensor("w_ih", [128, KE * G], bf16, kind="ExternalInput")
    w_hh = nc.dram_tensor("w_hh", [128, KH * G], bf16, kind="ExternalInput")
    b_g = nc.dram_tensor("b_g", [1, G], bf16, kind="ExternalInput")
    w_fc = nc.dram_tensor("w_fc", [128, KH * VL], bf16, kind="ExternalInput")
    b_fc = nc.dram_tensor("b_fc", [1, VL], bf16, kind="ExternalInput")
    ident = nc.dram_tensor("ident", [128, 128], f32r, kind="ExternalInput")
    ones = nc.dram_tensor("ones", [1, 128], bf16, kind="ExternalInput")
    out = nc.dram_tensor("out", [T * B, VL], f32, kind="ExternalOutput")

    with tile.TileContext(nc) as tc:
        with tc.tile_pool(name="wp", bufs=1) as wp, \
             tc.tile_pool(name="xg", bufs=1) as xg, \
             tc.tile_pool(name="work", bufs=2) as work, \
             tc.tile_pool(name="pxg", bufs=3, space="PSUM") as pxg, \
             tc.tile_pool(name="ptr", bufs=1, space="PSUM") as ptr, \
             tc.tile_pool(name="pfc", bufs=2, space="PSUM") as pfc, \
             tc.tile_pool(name="dp", bufs=1, space="DRAM") as dp:

            def load(src, shape, dtype, tag):
                t = wp.tile(shape, dtype, tag=tag)
                nc.scalar.dma_start(t[:], src[:])
                return t

            # --- minimal loads for the h0 publish path (AG0 asap) ---
            feat_sb = load(featT, [128, KF * B], bf16, "feat")
            wih0_sb = load(w_init_h, [128, KF * HC], bf16, "wih0")
            bih_sb = load(b_init_h, [1, HC], bf16, "bih")
            id_sb = load(ident, [128, 128], f32r, "id")
            on_sb = load(ones, [1, 128], bf16, "on")

            fcl_tiles = {}

            def publish(i, hk_tile):
                """hk_tile [B,HC] f32r -> transpose -> bf16 -> AllGather."""
                ps = ptr.tile([128, 128], f32r, tag="pstr")
                nc.tensor.transpose(ps[:, 0:B], hk_tile[:], id_sb[0:B, 0:B])
                hTm = work.tile([128, B], bf16, tag="hTm")
                nc.vector.tensor_copy(hTm[:], ps[:, 0:B])
                bi = dp.tile([128, B], bf16, tag=f"bi{i}")
                nc.sync.dma_start(bi[:], hTm[:])
                bo = dp.tile([KH * 128, B], bf16, tag=f"bo{i}")
                nc.gpsimd.collective_compute(
                    "AllGather", mybir.AluOpType.bypass, replica_groups=RG,
                    ins=[bi.opt()], outs=[bo.opt()],
                )
                hT = work.tile([128, KH * B], bf16, bufs=3, tag="hTall")
                half = KH // 2
                nc.sync.dma_start(
                    hT[:, 0:half * B].rearrange("p (c b) -> p c b", c=half),
                    bo[0:half * 128, :].rearrange("(c p) b -> p c b", p=128))
                nc.scalar.dma_start(
                    hT[:, half * B:].rearrange("p (c b) -> p c b", c=half),
                    bo[half * 128:, :].rearrange("(c p) b -> p c b", p=128))
                if i >= 1:
                    p, off = (i - 1) // 2, B * ((i - 1) % 2)
                    if off == 0:
                        fcl_tiles[p] = work.tile([128, KH * 128], bf16, bufs=3,
                                                 tag="fcl", name=f"fcl{p}")
                    dst = fcl_tiles[p][:].rearrange(
                        "p (c x) -> p c x", x=128)[:, :, off:off + B]
                    nc.scalar.dma_start(
                        dst, bo[:].rearrange("(c p) b -> p c b", p=128))
                return hT

            # ---- h0 (publish immediately) ----
            ps_h0 = pxg.tile([B, G], f32, tag="pxg")
            nc.tensor.matmul(ps_h0[:, 0:HC], on_sb[0:1, 0:B], bih_sb[0:1, :],
                             start=True, stop=False)
            for k in range(KF):
                nc.tensor.matmul(ps_h0[:, 0:HC],
                                 feat_sb[:, k * B:(k + 1) * B],
                                 wih0_sb[:, k * HC:(k + 1) * HC],
                                 start=False, stop=(k == KF - 1))
            hk0 = work.tile([B, HC], f32r, tag="hk")
            nc.vector.tensor_copy(hk0[:], ps_h0[:, 0:HC])
            hT = publish(0, hk0)

            # --- remaining loads (overlap AG0 window) ---
            wih_sb = load(w_ih, [128, KE * G], bf16, "wih")
            whh_sb = load(w_hh, [128, KH * G], bf16, "whh")
            wic0_sb = load(w_init_c, [128, KF * HC], bf16, "wic0")
            bic_sb = load(b_init_c, [1, HC], bf16, "bic")
            bg_sb = load(b_g, [1, G], bf16, "bg")
            wfc_sb = load(w_fc, [128, KH * VL], bf16, "wfc")
            bfc_sb = load(b_fc, [1, VL], bf16, "bfc")
            ix_sb = wp.tile([128, NG], i32, tag="ix")
            nc.scalar.dma_start(ix_sb[:], idx_g[:])
            c_sb = wp.tile([B, HC], f32, tag="c")

            # ---- c0 ----
            ps_c0 = pxg.tile([B, G], f32, tag="pxg")
            nc.tensor.matmul(ps_c0[:, 0:HC], on_sb[0:1, 0:B], bic_sb[0:1, :],
                             start=True, stop=False)
            for k in range(KF):
                nc.tensor.matmul(ps_c0[:, 0:HC],
                                 feat_sb[:, k * B:(k + 1) * B],
                                 wic0_sb[:, k * HC:(k + 1) * HC],
                                 start=False, stop=(k == KF - 1))
            nc.vector.tensor_copy(c_sb[:], ps_c0[:, 0:HC])

            # ---- embedding gather + transpose into x.T layout ----
            xT = {}

            def emit_emb_group(g):
                emb_t = work.tile([128, E], f32r, tag="embg")
                nc.gpsimd.indirect_dma_start(
                    out=emb_t[:], out_offset=None, in_=emb_tab[:],
                    in_offset=bass.IndirectOffsetOnAxis(
                        ap=ix_sb[:, g:g + 1], axis=0),
                )
                xg_t = xg.tile([128, KE * 128], bf16, tag=f"xT{g}",
                               name=f"xTt{g}")
                for e in range(KE):
                    ps = ptr.tile([128, 128], f32r, tag="pstr")
                    nc.tensor.transpose(ps[:], emb_t[:, e * 128:(e + 1) * 128],
                                        id_sb[:])
                    nc.vector.tensor_copy(xg_t[:, e * 128:(e + 1) * 128],
                                          ps[:])
                xT[g] = xg_t

            for g in range(3):
                emit_emb_group(g)

            # ---- x-part gate pre-accumulation for step s (2 ahead) ----
            xg_psum = {}

            def emit_xpart(s):
                ps_g = pxg.tile([B, G], f32, tag="pxg", name=f"pxg{s}")
                nc.tensor.matmul(ps_g[:], on_sb[0:1, 0:B], bg_sb[0:1, :],
                                 start=True, stop=False)
                xs = xT[s // 2]
                xoff = (s % 2) * B
                for e in range(KE):
                    nc.tensor.matmul(ps_g[:],
                                     xs[:, e * 128 + xoff:e * 128 + xoff + B],
                                     wih_sb[:, e * G:(e + 1) * G],
                                     start=False, stop=False)
                xg_psum[s] = ps_g

            emit_xpart(0)
            emit_xpart(1)

            # ---- fc group scheduling ----
            fc_pending = []

            def emit_fc_group(p, n, nsz):
                fcl = fcl_tiles[p]
                ps_f = pfc.tile([128, 512], f32, tag="psfc")
                nc.tensor.matmul(ps_f[:, 0:nsz], on_sb[0:1, 0:128],
                                 bfc_sb[0:1, n:n + nsz], start=True,
                                 stop=False)
                for c in range(KH):
                    nc.tensor.matmul(ps_f[:, 0:nsz],
                                     fcl[:, c * 128:(c + 1) * 128],
                                     wfc_sb[:, c * VL + n:c * VL + n + nsz],
                                     start=False, stop=(c == KH - 1))
                osb = work.tile([128, 512], f32, tag="osb")
                nc.scalar.activation(osb[:, 0:nsz], ps_f[:, 0:nsz], SIG)
                nc.gpsimd.dma_start(out[p * 128:(p + 1) * 128, n:n + nsz],
                                    osb[:, 0:nsz])

            # ---- recurrence ----
            for s in range(T):
                ps_g = xg_psum.pop(s)
                for c in range(KH):
                    nc.tensor.matmul(ps_g[:], hT[:, c * B:(c + 1) * B],
                                     whh_sb[:, c * G:(c + 1) * G],
                                     start=False, stop=(c == KH - 1))
                # g-rows of W/x/bias pre-scaled x2 at staging:
                # tanh(g) = 2*sigmoid(2g) - 1, all four gates in one sigmoid
                gsb = work.tile([B, G], f32, tag="gsb")
                nc.scalar.activation(gsb[:], ps_g[:], SIG)
                t1 = work.tile([B, HC], f32, tag="t1")
                nc.vector.scalar_tensor_tensor(
                    t1[:], gsb[:, 0:128], 2.0, gsb[:, 128:256],
                    op0=MUL, op1=MUL)                      # (2*sg)*i
                nc.vector.tensor_sub(t1[:], t1[:], gsb[:, 128:256])
                nc.vector.tensor_mul(c_sb[:], c_sb[:], gsb[:, 256:384])
                nc.vector.tensor_add(c_sb[:], c_sb[:], t1[:])
                thc = work.tile([B, HC], f32, tag="thc")
                nc.scalar.activation(thc[:], c_sb[:], SIG, scale=2.0)
                hk = work.tile([B, HC], f32r, tag="hk")
                nc.vector.scalar_tensor_tensor(
                    hk[:], thc[:], 2.0, gsb[:, 384:512],
                    op0=MUL, op1=MUL)                      # (2*sc)*o
                nc.vector.tensor_sub(hk[:], hk[:], gsb[:, 384:512])
                hT = publish(s + 1, hk)

                # window work: emb pipeline, x-part 2 ahead, fc groups
                g_next = (s + 5) // 2
                if s % 2 == 1 and g_next < NG:
                    emit_emb_group(g_next)
                if s + 2 < T:
                    emit_xpart(s + 2)
                if s % 2 == 1:
                    p = s // 2
                    fc_pending.extend(
                        (p, n, min(512, VL - n)) for n in range(0, VL, 512))
                npop = 1 if s % 2 == 1 else 2
                for _ in range(min(npop, len(fc_pending))):
                    emit_fc_group(*fc_pending.pop(0))

            while fc_pending:
                emit_fc_group(*fc_pending.pop(0))

    nc.compile()
    return nc


def _prep_inputs(features, captions, embed_table, W_init_h, b_init_h,
                 W_init_c, b_init_c, W_ih, b_ih, W_hh, b_hh, W_fc, b_fc):
    import ml_dtypes
    f = np.float32
    bf = ml_dtypes.bfloat16
    features = np.asarray(features, f)
    captions = np.asarray(captions)
    embed_table = np.asarray(embed_table, f)
    W_init_h, b_init_h = np.asarray(W_init_h, f), np.asarray(b_init_h, f)
    W_init_c, b_init_c = np.asarray(W_init_c, f), np.asarray(b_init_c, f)
    W_ih, b_ih = np.asarray(W_ih, f), np.asarray(b_ih, f)
    W_hh, b_hh = np.asarray(W_hh, f), np.asarray(b_hh, f)
    W_fc, b_fc = np.asarray(W_fc, f), np.asarray(b_fc, f)

    def chunked_T(m, kchunks):
        # m: [rows, cols] -> m.T laid out [128, kchunks*cols], bf16
        rows, cols = m.shape
        assert rows == kchunks * 128
        return np.ascontiguousarray(
            m.T.reshape(cols, kchunks, 128).transpose(2, 1, 0)
            .reshape(128, kchunks * cols)).astype(bf)

    r = np.arange(T * B)
    idx_flat = captions[r % B, r // B].astype(np.int32)
    idx_g = np.ascontiguousarray(idx_flat.reshape(NG, 128).T)

    featT = np.ascontiguousarray(
        features.T.reshape(KF, 128, B).transpose(1, 0, 2)
        .reshape(128, KF * B)).astype(bf)

    bg_full = b_ih + b_hh
    # pre-scale the g-gate rows by 2 (tanh-via-sigmoid trick)
    gsc = np.ones(4 * H, np.float32)
    gsc[2 * H:3 * H] = 2.0
    Wih_s = W_ih * gsc[:, None]
    Whh_s = W_hh * gsc[:, None]
    bg_s = bg_full * gsc
    Wfc_pad = np.zeros((VL * N_CORES, H), f)
    Wfc_pad[:V] = W_fc
    bfc_pad = np.zeros(VL * N_CORES, f)
    bfc_pad[:V] = b_fc

    ident = np.eye(128, dtype=f)
    ones = np.ones((1, 128), bf)

    in_maps = []
    for k in range(N_CORES):
        # gate slice rows in [g|i|f|o] order (reference packs i,f,g,o)
        rows = np.concatenate([
            np.arange(2 * H + 128 * k, 2 * H + 128 * (k + 1)),   # g
            np.arange(0 * H + 128 * k, 0 * H + 128 * (k + 1)),   # i
            np.arange(1 * H + 128 * k, 1 * H + 128 * (k + 1)),   # f
            np.arange(3 * H + 128 * k, 3 * H + 128 * (k + 1)),   # o
        ])
        hs = slice(128 * k, 128 * (k + 1))
        vs = slice(VL * k, VL * (k + 1))
        in_maps.append({
            "emb_tab": embed_table,
            "idx_g": idx_g,
            "featT": featT,
            "w_init_h": chunked_T(W_init_h[hs].T, KF),
            "w_init_c": chunked_T(W_init_c[hs].T, KF),
            "b_init_h": b_init_h[hs].reshape(1, HC).astype(bf),
            "b_init_c": b_init_c[hs].reshape(1, HC).astype(bf),
            "w_ih": chunked_T(Wih_s[rows].T, KE),
            "w_hh": chunked_T(Whh_s[rows].T, KH),
            "b_g": bg_s[rows].reshape(1, G).astype(bf),
            "w_fc": chunked_T(Wfc_pad[vs].T, KH),
            "b_fc": bfc_pad[vs].reshape(1, VL).astype(bf),
            "ident": ident,
            "ones": ones,
        })
    return in_maps


def kernel(**inputs):
    global _built, last_results
    from concourse import bass_utils
    if _built is None:
        _built = _build()
    in_maps = _prep_inputs(**inputs)
    res = bass_utils.run_bass_kernel_spmd(
        _built, in_maps, core_ids=list(range(N_CORES)))
    last_results = res
    chunks = [r["out"].reshape(T, B, VL) for r in res.results]
    return np.concatenate(chunks, axis=2)[:, :, :V].astype(np.float32)
